# revision 15
# baseline (speedup 1.0000x reference)
"""Bass/Tile TRN2 kernel for nn_BlockLearnableCompressionMatrix.

Computes, for x (B=16, 1, n=1024, T=2048) f32 and blocks (m=128, c=8) f32:
    w      = tanh(blocks)                          # (128, 8)
    Ax     = einsum('bmct,mc->bmt', x.reshape(B, 128, 8, T), w)   # (16, 128, 2048)
    A_full = block-diagonal expansion of w         # (128, 1024)

Sharding: pure data parallel over batch B across 8 NeuronCores (2 batches
per core); blocks replicated. Each core computes its Ax shard; A_full is
computed redundantly on every core and taken from core 0.

Per-core schedule (memory-bound; HBM ~358 GB/s/core is the roofline):
  - x streams in as 8 tiles of (128 partitions=m, 8=c, 512=t) on the sync
    HWDGE ring (2 MB each, 2 KB contiguous runs).
  - Per tile, the segment-reduce over c splits across two engines:
    ACT computes 4 products p_c = x_c * w[:,c] (per-partition scale),
    DVE does 4 fused multiply-adds (x_c*w[:,c] + p) and 3 pairwise adds.
  - Stores and ACT work share the scalar HWDGE ring; loads own sync.
  - A_full is written with two disjoint DMAs (gap runs + diagonal blocks)
    hidden under the first x load.
"""

import os
import sys
import types

import numpy as np

# ---------------------------------------------------------------------------
# NTFF profile hook injection: the trimmed container's `antenv` package lacks
# `axon_hooks`, which bass_utils needs for trace=True under axon. Register the
# ctypes-based hook from trn_agent_boot so profiling works when requested.
# Harmless (and skipped) when unavailable or when tracing is never requested.
try:
    import antenv
    from trn_agent_boot.trn_boot import _ntff_profile_via_ctypes

    if "antenv.axon_hooks" not in sys.modules:
        _hook = _ntff_profile_via_ctypes("/opt/axon/libaxon_pjrt.so")
        _mod = types.ModuleType("antenv.axon_hooks")
        _mod.get_axon_ntff_profile_hook = lambda: _hook
        sys.modules["antenv.axon_hooks"] = _mod
        antenv.axon_hooks = _mod
except Exception:
    pass

import concourse.bass as bass
import concourse.bacc as bacc
import concourse.mybir as mybir
import concourse.tile as tile
from concourse.bass_utils import run_bass_kernel_spmd

N_CORES = 8
B = 16            # full batch
B_SH = B // N_CORES   # batches per core
M = 128           # number of blocks == partitions
C = 8             # block size
N_CH = M * C      # 1024 input channels
T = 2048          # time dim
# Asymmetric t-tiling per batch: small first tiles so compute starts early
# (SDMA round-robins queued transfers, so first-tile latency ~ sum of
# co-queued bytes), small last tiles so the compute tail after the final
# load is short, 512-wide in steady state.
TILES_B0 = [128, 384, 512, 512, 512]
TILES_B1 = [512, 512, 512, 384, 128]

F32 = mybir.dt.float32
MULT = None  # set after import
ADD = None

# Results of the last run (BassKernelResults), for test harnesses that want
# exec_time_ns from a traced run.
LAST_RESULTS = None

_COMPILED = None  # cached bass module


def _build_module():
    mult = mybir.AluOpType.mult
    add = mybir.AluOpType.add
    nc = bacc.Bacc(
        "TRN2",
        target_bir_lowering=False,
        debug=False,
        enable_asserts=False,
        num_devices=N_CORES,
    )
    x = nc.dram_tensor("x", [B_SH, N_CH, T], F32, kind="ExternalInput").ap()
    blocks = nc.dram_tensor("blocks", [M, C], F32, kind="ExternalInput").ap()
    ax = nc.dram_tensor("ax", [B_SH, M, T], F32, kind="ExternalOutput").ap()
    afull = nc.dram_tensor("afull", [M, N_CH], F32, kind="ExternalOutput").ap()

    # x viewed as (b, m, c, t): channel ch = m*8 + c
    xg = x.rearrange("b (m c) t -> b m c t", c=C)

    with tile.TileContext(nc) as tc:
        with (
            tc.tile_pool(name="consts", bufs=1) as cpool,
            tc.tile_pool(name="xin", bufs=6) as xpool,
            tc.tile_pool(name="prod", bufs=3) as ppool,
            tc.tile_pool(name="acc", bufs=3) as apool,
            tc.tile_pool(name="out", bufs=2) as opool,
        ):
            # --- weights: w = tanh(blocks). Load blocks as ONE flat
            # descriptor (a (128,8) load would be 128 x 32-byte descriptors,
            # each paying an HBM round trip), then spread to per-partition
            # layout with a local SBUF->SBUF DMA (cheap), then tanh.
            blocks_flat = cpool.tile([1, M * C], F32)
            nc.sync.dma_start(
                blocks_flat[:], blocks.rearrange("m c -> (m c)").unsqueeze(0)
            )
            blocks_sb = cpool.tile([M, C], F32)
            nc.sync.dma_start(
                blocks_sb[:], blocks_flat[:].rearrange("o (m c) -> o m c", c=C)
            )
            w_sb = cpool.tile([M, C], F32)
            nc.scalar.activation(
                w_sb[:], blocks_sb[:], mybir.ActivationFunctionType.Tanh
            )

            # --- A_full: build the row image [w_m | 0...0] (1032 wide) in
            # SBUF and write it over the flat DRAM layout, which is exactly
            # [diag_0 (8) | gap_0 (1024) | diag_1 (8) | ... | diag_127 (8)].
            # Partition m covers flat [m*1032, (m+1)*1032) for m < 127 (fat
            # 4128-byte descriptors); partition 127 contributes only its
            # 8-element diagonal tail. Runs early, hidden under x loads.
            img = cpool.tile([M, 1032], F32)
            nc.gpsimd.memset(img[:], 0.0)
            nc.scalar.activation(
                img[:, 0:C], w_sb[:], mybir.ActivationFunctionType.Copy
            )
            body = afull.copy()
            _ap = body.ap
            _ap[0] = [1032, 127]
            _ap[1] = [1, 1032]
            body.ap = _ap
            nc.gpsimd.dma_start(body, img[0:127, :])
            tail = afull.copy()
            tail.offset = 127 * 1032
            _ap = tail.ap
            _ap[0] = [8, 1]
            _ap[1] = [1, 8]
            tail.ap = _ap
            nc.gpsimd.dma_start(tail, img[127:128, 0:C])

            # --- main loop over (batch, t-tile) tiles. The final fold of
            # each tile lands in a per-batch (128, 2048) output tile; each
            # batch is stored with ONE fat-descriptor DMA (8 KB contiguous
            # per partition row) on the otherwise-idle SWDGE queue, keeping
            # small writes from thrashing the load stream.
            for b, widths in ((0, TILES_B0), (1, TILES_B1)):
                out_b = opool.tile([M, T], F32, tag="out")
                t0 = 0
                for w_t in widths:
                    ts = slice(t0, t0 + w_t)
                    t0 += w_t
                    xt = xpool.tile([M, C, w_t], F32, tag="xt")
                    nc.sync.dma_start(xt[:], xg[b, :, :, ts])

                    # ACT: products for c = 4..7
                    prods = []
                    for c in range(C // 2, C):
                        p = ppool.tile([M, w_t], F32, tag=f"p{c}")
                        nc.scalar.activation(
                            p[:],
                            xt[:, c, :],
                            mybir.ActivationFunctionType.Copy,
                            scale=w_sb[:, c : c + 1],
                        )
                        prods.append(p)

                    # DVE: a_i = x_i*w_i + p_{i+4}, then pairwise fold
                    parts = []
                    for c in range(C // 2):
                        a = apool.tile([M, w_t], F32, tag=f"a{c}")
                        nc.vector.scalar_tensor_tensor(
                            a[:],
                            xt[:, c, :],
                            w_sb[:, c : c + 1],
                            prods[c][:],
                            op0=mult,
                            op1=add,
                        )
                        parts.append(a)
                    nc.vector.tensor_add(parts[0][:], parts[0][:], parts[1][:])
                    nc.vector.tensor_add(parts[2][:], parts[2][:], parts[3][:])
                    nc.vector.tensor_add(out_b[:, ts], parts[0][:], parts[2][:])
                nc.gpsimd.dma_start(ax[b], out_b[:])

    nc.compile()
    return nc


def kernel(x: np.ndarray, blocks: np.ndarray):
    """Full inputs in, full outputs out. Shards batch across 8 cores."""
    global LAST_RESULTS, _COMPILED
    if _COMPILED is None:
        _COMPILED = _build_module()
    nc = _COMPILED

    x = np.asarray(x, dtype=np.float32)
    blocks_np = np.ascontiguousarray(np.asarray(blocks, dtype=np.float32))
    if x.ndim == 4:
        x = x[:, 0]
    in_maps = [
        {
            "x": np.ascontiguousarray(x[k * B_SH : (k + 1) * B_SH]),
            "blocks": blocks_np,
        }
        for k in range(N_CORES)
    ]
    res = run_bass_kernel_spmd(nc, in_maps, core_ids=list(range(N_CORES)))
    LAST_RESULTS = res
    ax = np.concatenate([res.results[k]["ax"] for k in range(N_CORES)], axis=0)
    a_full = res.results[0]["afull"]
    return ax, a_full


# revision 17
# speedup vs baseline: 1.0154x; 1.0154x over previous
"""Bass/Tile TRN2 kernel for nn_BlockLearnableCompressionMatrix.

Computes, for x (B=16, 1, n=1024, T=2048) f32 and blocks (m=128, c=8) f32:
    w      = tanh(blocks)                          # (128, 8)
    Ax     = einsum('bmct,mc->bmt', x.reshape(B, 128, 8, T), w)   # (16, 128, 2048)
    A_full = block-diagonal expansion of w         # (128, 1024)

Sharding: pure data parallel over batch B across 8 NeuronCores (2 batches
per core); blocks replicated. Each core computes its Ax shard; A_full is
computed redundantly on every core and taken from core 0.

Per-core schedule (memory-bound; HBM ~358 GB/s/core is the roofline):
  - x streams in as 8 tiles of (128 partitions=m, 8=c, 512=t) on the sync
    HWDGE ring (2 MB each, 2 KB contiguous runs).
  - Per tile, the segment-reduce over c splits across two engines:
    ACT computes 4 products p_c = x_c * w[:,c] (per-partition scale),
    DVE does 4 fused multiply-adds (x_c*w[:,c] + p) and 3 pairwise adds.
  - Stores and ACT work share the scalar HWDGE ring; loads own sync.
  - A_full is written with two disjoint DMAs (gap runs + diagonal blocks)
    hidden under the first x load.
"""

import os
import sys
import types

import numpy as np

# ---------------------------------------------------------------------------
# NTFF profile hook injection: the trimmed container's `antenv` package lacks
# `axon_hooks`, which bass_utils needs for trace=True under axon. Register the
# ctypes-based hook from trn_agent_boot so profiling works when requested.
# Harmless (and skipped) when unavailable or when tracing is never requested.
try:
    import antenv
    from trn_agent_boot.trn_boot import _ntff_profile_via_ctypes

    if "antenv.axon_hooks" not in sys.modules:
        _hook = _ntff_profile_via_ctypes("/opt/axon/libaxon_pjrt.so")
        _mod = types.ModuleType("antenv.axon_hooks")
        _mod.get_axon_ntff_profile_hook = lambda: _hook
        sys.modules["antenv.axon_hooks"] = _mod
        antenv.axon_hooks = _mod
except Exception:
    pass

import concourse.bass as bass
import concourse.bacc as bacc
import concourse.mybir as mybir
import concourse.tile as tile
from concourse.bass_utils import run_bass_kernel_spmd

N_CORES = 8
B = 16            # full batch
B_SH = B // N_CORES   # batches per core
M = 128           # number of blocks == partitions
C = 8             # block size
N_CH = M * C      # 1024 input channels
T = 2048          # time dim
# Asymmetric t-tiling per batch: small first tiles so compute starts early
# (SDMA round-robins queued transfers, so first-tile latency ~ sum of
# co-queued bytes), small last tiles so the compute tail after the final
# load is short, 512-wide in steady state.
TILES_B0 = [256, 768, 768, 256]
TILES_B1 = [768, 768, 384, 128]

F32 = mybir.dt.float32
MULT = None  # set after import
ADD = None

# Results of the last run (BassKernelResults), for test harnesses that want
# exec_time_ns from a traced run.
LAST_RESULTS = None

_COMPILED = None  # cached bass module


def _build_module():
    mult = mybir.AluOpType.mult
    add = mybir.AluOpType.add
    nc = bacc.Bacc(
        "TRN2",
        target_bir_lowering=False,
        debug=False,
        enable_asserts=False,
        num_devices=N_CORES,
    )
    x = nc.dram_tensor("x", [B_SH, N_CH, T], F32, kind="ExternalInput").ap()
    blocks = nc.dram_tensor("blocks", [M, C], F32, kind="ExternalInput").ap()
    ax = nc.dram_tensor("ax", [B_SH, M, T], F32, kind="ExternalOutput").ap()
    afull = nc.dram_tensor("afull", [M, N_CH], F32, kind="ExternalOutput").ap()

    # x viewed as (b, m, c, t): channel ch = m*8 + c
    xg = x.rearrange("b (m c) t -> b m c t", c=C)

    with tile.TileContext(nc) as tc:
        with (
            tc.tile_pool(name="consts", bufs=1) as cpool,
            tc.tile_pool(name="xin", bufs=4) as xpool,
            tc.tile_pool(name="prod", bufs=2) as ppool,
            tc.tile_pool(name="acc", bufs=2) as apool,
            tc.tile_pool(name="out", bufs=2) as opool,
        ):
            # --- weights: w = tanh(blocks). Load blocks as ONE flat
            # descriptor (a (128,8) load would be 128 x 32-byte descriptors,
            # each paying an HBM round trip), then spread to per-partition
            # layout with a local SBUF->SBUF DMA (cheap), then tanh.
            blocks_flat = cpool.tile([1, M * C], F32)
            nc.sync.dma_start(
                blocks_flat[:], blocks.rearrange("m c -> (m c)").unsqueeze(0)
            )
            blocks_sb = cpool.tile([M, C], F32)
            nc.gpsimd.dma_start(
                blocks_sb[:], blocks_flat[:].rearrange("o (m c) -> o m c", c=C)
            )
            w_sb = cpool.tile([M, C], F32)
            nc.scalar.activation(
                w_sb[:], blocks_sb[:], mybir.ActivationFunctionType.Tanh
            )

            # --- A_full: build the row image [w_m | 0...0] (1032 wide) in
            # SBUF and write it over the flat DRAM layout, which is exactly
            # [diag_0 (8) | gap_0 (1024) | diag_1 (8) | ... | diag_127 (8)].
            # Partition m covers flat [m*1032, (m+1)*1032) for m < 127 (fat
            # 4128-byte descriptors); partition 127 contributes only its
            # 8-element diagonal tail. Runs early, hidden under x loads.
            img = cpool.tile([M, 1032], F32)
            nc.gpsimd.memset(img[:], 0.0)
            nc.scalar.activation(
                img[:, 0:C], w_sb[:], mybir.ActivationFunctionType.Copy
            )
            body = afull.copy()
            _ap = body.ap
            _ap[0] = [1032, 127]
            _ap[1] = [1, 1032]
            body.ap = _ap
            nc.gpsimd.dma_start(body, img[0:127, :])
            tail = afull.copy()
            tail.offset = 127 * 1032
            _ap = tail.ap
            _ap[0] = [8, 1]
            _ap[1] = [1, 8]
            tail.ap = _ap
            nc.gpsimd.dma_start(tail, img[127:128, 0:C])

            # --- main loop over (batch, t-tile) tiles. The final fold of
            # each tile lands in a per-batch (128, 2048) output tile; each
            # batch is stored with ONE fat-descriptor DMA (8 KB contiguous
            # per partition row) on the otherwise-idle SWDGE queue, keeping
            # small writes from thrashing the load stream.
            for b, widths in ((0, TILES_B0), (1, TILES_B1)):
                out_b = opool.tile([M, T], F32, tag="out")
                t0 = 0
                stored = 0
                for w_t in widths:
                    ts = slice(t0, t0 + w_t)
                    t0 += w_t
                    xt = xpool.tile([M, C, w_t], F32, tag="xt")
                    nc.sync.dma_start(xt[:], xg[b, :, :, ts])

                    # ACT: products for c = 4..7
                    prods = []
                    for c in range(C // 2, C):
                        p = ppool.tile([M, w_t], F32, tag=f"p{c}")
                        nc.scalar.activation(
                            p[:],
                            xt[:, c, :],
                            mybir.ActivationFunctionType.Copy,
                            scale=w_sb[:, c : c + 1],
                        )
                        prods.append(p)

                    # DVE: a_i = x_i*w_i + p_{i+4}, then pairwise fold
                    parts = []
                    for c in range(C // 2):
                        a = apool.tile([M, w_t], F32, tag=f"a{c}")
                        nc.vector.scalar_tensor_tensor(
                            a[:],
                            xt[:, c, :],
                            w_sb[:, c : c + 1],
                            prods[c][:],
                            op0=mult,
                            op1=add,
                        )
                        parts.append(a)
                    nc.vector.tensor_add(parts[0][:], parts[0][:], parts[1][:])
                    nc.vector.tensor_add(parts[2][:], parts[2][:], parts[3][:])
                    nc.vector.tensor_add(out_b[:, ts], parts[0][:], parts[2][:])
                    if stored == 0 and t0 >= T // 2:
                        nc.gpsimd.dma_start(
                            ax[b, :, 0 : T // 2], out_b[:, 0 : T // 2]
                        )
                        stored = t0
                nc.gpsimd.dma_start(ax[b, :, T // 2 :], out_b[:, T // 2 :])

    nc.compile()
    return nc


def kernel(x: np.ndarray, blocks: np.ndarray):
    """Full inputs in, full outputs out. Shards batch across 8 cores."""
    global LAST_RESULTS, _COMPILED
    if _COMPILED is None:
        _COMPILED = _build_module()
    nc = _COMPILED

    x = np.asarray(x, dtype=np.float32)
    blocks_np = np.ascontiguousarray(np.asarray(blocks, dtype=np.float32))
    if x.ndim == 4:
        x = x[:, 0]
    in_maps = [
        {
            "x": np.ascontiguousarray(x[k * B_SH : (k + 1) * B_SH]),
            "blocks": blocks_np,
        }
        for k in range(N_CORES)
    ]
    res = run_bass_kernel_spmd(nc, in_maps, core_ids=list(range(N_CORES)))
    LAST_RESULTS = res
    ax = np.concatenate([res.results[k]["ax"] for k in range(N_CORES)], axis=0)
    a_full = res.results[0]["afull"]
    return ax, a_full


# revision 18
# speedup vs baseline: 1.0294x; 1.0139x over previous
"""Bass/Tile TRN2 kernel for nn_BlockLearnableCompressionMatrix.

Computes, for x (B=16, 1, n=1024, T=2048) f32 and blocks (m=128, c=8) f32:
    w      = tanh(blocks)                          # (128, 8)
    Ax     = einsum('bmct,mc->bmt', x.reshape(B, 128, 8, T), w)   # (16, 128, 2048)
    A_full = block-diagonal expansion of w         # (128, 1024)

Sharding: pure data parallel over batch B across 8 NeuronCores (2 batches
per core); blocks replicated. Each core computes its Ax shard; A_full is
computed redundantly on every core and taken from core 0.

Per-core schedule (memory-bound; HBM ~358 GB/s/core is the roofline):
  - x streams in as 8 tiles of (128 partitions=m, 8=c, 512=t) on the sync
    HWDGE ring (2 MB each, 2 KB contiguous runs).
  - Per tile, the segment-reduce over c splits across two engines:
    ACT computes 4 products p_c = x_c * w[:,c] (per-partition scale),
    DVE does 4 fused multiply-adds (x_c*w[:,c] + p) and 3 pairwise adds.
  - Stores and ACT work share the scalar HWDGE ring; loads own sync.
  - A_full is written with two disjoint DMAs (gap runs + diagonal blocks)
    hidden under the first x load.
"""

import os
import sys
import types

import numpy as np

# ---------------------------------------------------------------------------
# NTFF profile hook injection: the trimmed container's `antenv` package lacks
# `axon_hooks`, which bass_utils needs for trace=True under axon. Register the
# ctypes-based hook from trn_agent_boot so profiling works when requested.
# Harmless (and skipped) when unavailable or when tracing is never requested.
try:
    import antenv
    from trn_agent_boot.trn_boot import _ntff_profile_via_ctypes

    if "antenv.axon_hooks" not in sys.modules:
        _hook = _ntff_profile_via_ctypes("/opt/axon/libaxon_pjrt.so")
        _mod = types.ModuleType("antenv.axon_hooks")
        _mod.get_axon_ntff_profile_hook = lambda: _hook
        sys.modules["antenv.axon_hooks"] = _mod
        antenv.axon_hooks = _mod
except Exception:
    pass

import concourse.bass as bass
import concourse.bacc as bacc
import concourse.mybir as mybir
import concourse.tile as tile
from concourse.bass_utils import run_bass_kernel_spmd

N_CORES = 8
B = 16            # full batch
B_SH = B // N_CORES   # batches per core
M = 128           # number of blocks == partitions
C = 8             # block size
N_CH = M * C      # 1024 input channels
T = 2048          # time dim
# Asymmetric t-tiling per batch: small first tiles so compute starts early
# (SDMA round-robins queued transfers, so first-tile latency ~ sum of
# co-queued bytes), small last tiles so the compute tail after the final
# load is short, 512-wide in steady state.
TILES_B0 = [256, 768, 768, 256]
TILES_B1 = [768, 768, 384, 128]

F32 = mybir.dt.float32
MULT = None  # set after import
ADD = None

# Results of the last run (BassKernelResults), for test harnesses that want
# exec_time_ns from a traced run.
LAST_RESULTS = None

_COMPILED = None  # cached bass module


def _build_module():
    mult = mybir.AluOpType.mult
    add = mybir.AluOpType.add
    nc = bacc.Bacc(
        "TRN2",
        target_bir_lowering=False,
        debug=False,
        enable_asserts=False,
        num_devices=N_CORES,
    )
    x = nc.dram_tensor("x", [B_SH, N_CH, T], F32, kind="ExternalInput").ap()
    blocks = nc.dram_tensor("blocks", [M, C], F32, kind="ExternalInput").ap()
    ax = nc.dram_tensor("ax", [B_SH, M, T], F32, kind="ExternalOutput").ap()
    afull = nc.dram_tensor("afull", [M, N_CH], F32, kind="ExternalOutput").ap()

    # x viewed as (b, m, c, t): channel ch = m*8 + c
    xg = x.rearrange("b (m c) t -> b m c t", c=C)

    with tile.TileContext(nc) as tc:
        with (
            tc.tile_pool(name="consts", bufs=1) as cpool,
            tc.tile_pool(name="xin", bufs=5) as xpool,
            tc.tile_pool(name="prod", bufs=2) as ppool,
            tc.tile_pool(name="acc", bufs=2) as apool,
            tc.tile_pool(name="out", bufs=2) as opool,
        ):
            # --- weights: w = tanh(blocks). Load blocks as ONE flat
            # descriptor (a (128,8) load would be 128 x 32-byte descriptors,
            # each paying an HBM round trip), then spread to per-partition
            # layout with a local SBUF->SBUF DMA (cheap), then tanh.
            blocks_flat = cpool.tile([1, M * C], F32)
            nc.scalar.dma_start(
                blocks_flat[:], blocks.rearrange("m c -> (m c)").unsqueeze(0)
            )
            blocks_sb = cpool.tile([M, C], F32)
            nc.scalar.dma_start(
                blocks_sb[:], blocks_flat[:].rearrange("o (m c) -> o m c", c=C)
            )
            w_sb = cpool.tile([M, C], F32)
            nc.scalar.activation(
                w_sb[:], blocks_sb[:], mybir.ActivationFunctionType.Tanh
            )

            # --- A_full: build the row image [w_m | 0...0] (1032 wide) in
            # SBUF and write it over the flat DRAM layout, which is exactly
            # [diag_0 (8) | gap_0 (1024) | diag_1 (8) | ... | diag_127 (8)].
            # Partition m covers flat [m*1032, (m+1)*1032) for m < 127 (fat
            # 4128-byte descriptors); partition 127 contributes only its
            # 8-element diagonal tail. Runs early, hidden under x loads.
            img = cpool.tile([M, 1032], F32)
            nc.gpsimd.memset(img[:], 0.0)
            nc.scalar.activation(
                img[:, 0:C], w_sb[:], mybir.ActivationFunctionType.Copy
            )
            body = afull.copy()
            _ap = body.ap
            _ap[0] = [1032, 127]
            _ap[1] = [1, 1032]
            body.ap = _ap
            nc.gpsimd.dma_start(body, img[0:127, :])
            tail = afull.copy()
            tail.offset = 127 * 1032
            _ap = tail.ap
            _ap[0] = [8, 1]
            _ap[1] = [1, 8]
            tail.ap = _ap
            nc.gpsimd.dma_start(tail, img[127:128, 0:C])

            # --- main loop over (batch, t-tile) tiles. The final fold of
            # each tile lands in a per-batch (128, 2048) output tile; each
            # batch is stored with ONE fat-descriptor DMA (8 KB contiguous
            # per partition row) on the otherwise-idle SWDGE queue, keeping
            # small writes from thrashing the load stream.
            for b, widths in ((0, TILES_B0), (1, TILES_B1)):
                out_b = opool.tile([M, T], F32, tag="out")
                t0 = 0
                stored = 0
                for w_t in widths:
                    ts = slice(t0, t0 + w_t)
                    t0 += w_t
                    xt = xpool.tile([M, C, w_t], F32, tag="xt")
                    nc.sync.dma_start(xt[:], xg[b, :, :, ts])

                    # ACT: products for c = 4..7
                    prods = []
                    for c in range(C // 2, C):
                        p = ppool.tile([M, w_t], F32, tag=f"p{c}")
                        nc.scalar.activation(
                            p[:],
                            xt[:, c, :],
                            mybir.ActivationFunctionType.Copy,
                            scale=w_sb[:, c : c + 1],
                        )
                        prods.append(p)

                    # DVE: a_i = x_i*w_i + p_{i+4}, then pairwise fold
                    parts = []
                    for c in range(C // 2):
                        a = apool.tile([M, w_t], F32, tag=f"a{c}")
                        nc.vector.scalar_tensor_tensor(
                            a[:],
                            xt[:, c, :],
                            w_sb[:, c : c + 1],
                            prods[c][:],
                            op0=mult,
                            op1=add,
                        )
                        parts.append(a)
                    nc.vector.tensor_add(parts[0][:], parts[0][:], parts[1][:])
                    nc.vector.tensor_add(parts[2][:], parts[2][:], parts[3][:])
                    nc.vector.tensor_add(out_b[:, ts], parts[0][:], parts[2][:])
                    if stored == 0 and t0 >= T // 2:
                        eng = nc.scalar if b == 0 else nc.sync
                        eng.dma_start(
                            ax[b, :, 0 : T // 2], out_b[:, 0 : T // 2]
                        )
                        stored = t0
                eng = nc.scalar if b == 0 else nc.sync
                eng.dma_start(ax[b, :, T // 2 :], out_b[:, T // 2 :])

    nc.compile()
    return nc


def kernel(x: np.ndarray, blocks: np.ndarray):
    """Full inputs in, full outputs out. Shards batch across 8 cores."""
    global LAST_RESULTS, _COMPILED
    if _COMPILED is None:
        _COMPILED = _build_module()
    nc = _COMPILED

    x = np.asarray(x, dtype=np.float32)
    blocks_np = np.ascontiguousarray(np.asarray(blocks, dtype=np.float32))
    if x.ndim == 4:
        x = x[:, 0]
    in_maps = [
        {
            "x": np.ascontiguousarray(x[k * B_SH : (k + 1) * B_SH]),
            "blocks": blocks_np,
        }
        for k in range(N_CORES)
    ]
    res = run_bass_kernel_spmd(nc, in_maps, core_ids=list(range(N_CORES)))
    LAST_RESULTS = res
    ax = np.concatenate([res.results[k]["ax"] for k in range(N_CORES)], axis=0)
    a_full = res.results[0]["afull"]
    return ax, a_full


# revision 20
# speedup vs baseline: 1.0345x; 1.0049x over previous
"""Bass/Tile TRN2 kernel for nn_BlockLearnableCompressionMatrix.

Computes, for x (B=16, 1, n=1024, T=2048) f32 and blocks (m=128, c=8) f32:
    w      = tanh(blocks)                          # (128, 8)
    Ax     = einsum('bmct,mc->bmt', x.reshape(B, 128, 8, T), w)   # (16, 128, 2048)
    A_full = block-diagonal expansion of w         # (128, 1024)

Sharding: pure data parallel over batch B across 8 NeuronCores (2 batches
per core); blocks replicated. Each core computes its Ax shard; A_full is
computed redundantly on every core and taken from core 0.

Per-core schedule (memory-bound; HBM ~358 GB/s/core is the roofline):
  - x streams in as 8 tiles of (128 partitions=m, 8=c, 512=t) on the sync
    HWDGE ring (2 MB each, 2 KB contiguous runs).
  - Per tile, the segment-reduce over c splits across two engines:
    ACT computes 4 products p_c = x_c * w[:,c] (per-partition scale),
    DVE does 4 fused multiply-adds (x_c*w[:,c] + p) and 3 pairwise adds.
  - Stores and ACT work share the scalar HWDGE ring; loads own sync.
  - A_full is written with two disjoint DMAs (gap runs + diagonal blocks)
    hidden under the first x load.
"""

import os
import sys
import types

import numpy as np

# ---------------------------------------------------------------------------
# NTFF profile hook injection: the trimmed container's `antenv` package lacks
# `axon_hooks`, which bass_utils needs for trace=True under axon. Register the
# ctypes-based hook from trn_agent_boot so profiling works when requested.
# Harmless (and skipped) when unavailable or when tracing is never requested.
try:
    import antenv
    from trn_agent_boot.trn_boot import _ntff_profile_via_ctypes

    if "antenv.axon_hooks" not in sys.modules:
        _hook = _ntff_profile_via_ctypes("/opt/axon/libaxon_pjrt.so")
        _mod = types.ModuleType("antenv.axon_hooks")
        _mod.get_axon_ntff_profile_hook = lambda: _hook
        sys.modules["antenv.axon_hooks"] = _mod
        antenv.axon_hooks = _mod
except Exception:
    pass

import concourse.bass as bass
import concourse.bacc as bacc
import concourse.mybir as mybir
import concourse.tile as tile
from concourse.bass_utils import run_bass_kernel_spmd

N_CORES = 8
B = 16            # full batch
B_SH = B // N_CORES   # batches per core
M = 128           # number of blocks == partitions
C = 8             # block size
N_CH = M * C      # 1024 input channels
T = 2048          # time dim
# Asymmetric t-tiling per batch: small first tiles so compute starts early
# (SDMA round-robins queued transfers, so first-tile latency ~ sum of
# co-queued bytes), small last tiles so the compute tail after the final
# load is short, 512-wide in steady state.
TILES_B0 = [256, 512, 512, 512, 256]
TILES_B1 = [512, 512, 512, 384, 128]

F32 = mybir.dt.float32
MULT = None  # set after import
ADD = None

# Results of the last run (BassKernelResults), for test harnesses that want
# exec_time_ns from a traced run.
LAST_RESULTS = None

_COMPILED = None  # cached bass module


def _build_module():
    mult = mybir.AluOpType.mult
    add = mybir.AluOpType.add
    nc = bacc.Bacc(
        "TRN2",
        target_bir_lowering=False,
        debug=False,
        enable_asserts=False,
        num_devices=N_CORES,
    )
    x = nc.dram_tensor("x", [B_SH, N_CH, T], F32, kind="ExternalInput").ap()
    blocks = nc.dram_tensor("blocks", [M, C], F32, kind="ExternalInput").ap()
    ax = nc.dram_tensor("ax", [B_SH, M, T], F32, kind="ExternalOutput").ap()
    afull = nc.dram_tensor("afull", [M, N_CH], F32, kind="ExternalOutput").ap()

    # x viewed as (b, m, c, t): channel ch = m*8 + c
    xg = x.rearrange("b (m c) t -> b m c t", c=C)

    with tile.TileContext(nc) as tc:
        with (
            tc.tile_pool(name="consts", bufs=1) as cpool,
            tc.tile_pool(name="xin", bufs=1) as xpool,
            tc.tile_pool(name="prod", bufs=2) as ppool,
            tc.tile_pool(name="acc", bufs=2) as apool,
            tc.tile_pool(name="out", bufs=2) as opool,
        ):
            # --- weights: w = tanh(blocks). Load blocks as ONE flat
            # descriptor (a (128,8) load would be 128 x 32-byte descriptors,
            # each paying an HBM round trip), then spread to per-partition
            # layout with a local SBUF->SBUF DMA (cheap), then tanh.
            blocks_flat = cpool.tile([1, M * C], F32)
            nc.scalar.dma_start(
                blocks_flat[:], blocks.rearrange("m c -> (m c)").unsqueeze(0)
            )
            blocks_sb = cpool.tile([M, C], F32)
            nc.scalar.dma_start(
                blocks_sb[:], blocks_flat[:].rearrange("o (m c) -> o m c", c=C)
            )
            w_sb = cpool.tile([M, C], F32)
            nc.scalar.activation(
                w_sb[:], blocks_sb[:], mybir.ActivationFunctionType.Tanh
            )

            # --- A_full: build the row image [w_m | 0...0] (1032 wide) in
            # SBUF and write it over the flat DRAM layout, which is exactly
            # [diag_0 (8) | gap_0 (1024) | diag_1 (8) | ... | diag_127 (8)].
            # Partition m covers flat [m*1032, (m+1)*1032) for m < 127 (fat
            # 4128-byte descriptors); partition 127 contributes only its
            # 8-element diagonal tail. Runs early, hidden under x loads.
            img = cpool.tile([M, 1032], F32)
            nc.gpsimd.memset(img[:], 0.0)
            nc.scalar.activation(
                img[:, 0:C], w_sb[:], mybir.ActivationFunctionType.Copy
            )
            body = afull.copy()
            _ap = body.ap
            _ap[0] = [1032, 127]
            _ap[1] = [1, 1032]
            body.ap = _ap
            nc.gpsimd.dma_start(body, img[0:127, :])
            tail = afull.copy()
            tail.offset = 127 * 1032
            _ap = tail.ap
            _ap[0] = [8, 1]
            _ap[1] = [1, 8]
            tail.ap = _ap
            nc.gpsimd.dma_start(tail, img[127:128, 0:C])

            # --- main loop. The entire per-core x shard (16 MB = 128 KB per
            # partition) stays resident in ONE big SBUF tile, so x loads
            # never wait on buffer recycling: the sync queue streams all
            # pieces back-to-back at full HBM rate and compute trails at its
            # own pace against slices of the resident tile.
            xbig = xpool.tile([M, C, B_SH * T], F32)
            for b, widths in ((0, TILES_B0), (1, TILES_B1)):
                out_b = opool.tile([M, T], F32, tag="out")
                t0 = 0
                stored = 0
                for w_t in widths:
                    ts = slice(t0, t0 + w_t)
                    bts = slice(b * T + t0, b * T + t0 + w_t)
                    t0 += w_t
                    xt = xbig[:, :, bts]
                    nc.sync.dma_start(xt, xg[b, :, :, ts])

                    # ACT: products for c = 4..7
                    prods = []
                    for c in range(C // 2, C):
                        p = ppool.tile([M, w_t], F32, tag=f"p{c}")
                        nc.scalar.activation(
                            p[:],
                            xt[:, c, :],
                            mybir.ActivationFunctionType.Copy,
                            scale=w_sb[:, c : c + 1],
                        )
                        prods.append(p)

                    # DVE: a_i = x_i*w_i + p_{i+4}, then pairwise fold
                    parts = []
                    for c in range(C // 2):
                        a = apool.tile([M, w_t], F32, tag=f"a{c}")
                        nc.vector.scalar_tensor_tensor(
                            a[:],
                            xt[:, c, :],
                            w_sb[:, c : c + 1],
                            prods[c][:],
                            op0=mult,
                            op1=add,
                        )
                        parts.append(a)
                    nc.vector.tensor_add(parts[0][:], parts[0][:], parts[1][:])
                    nc.vector.tensor_add(parts[2][:], parts[2][:], parts[3][:])
                    nc.vector.tensor_add(out_b[:, ts], parts[0][:], parts[2][:])
                    if stored == 0 and t0 >= T // 2:
                        eng = nc.scalar if b == 0 else nc.sync
                        eng.dma_start(
                            ax[b, :, 0 : T // 2], out_b[:, 0 : T // 2]
                        )
                        stored = t0
                eng = nc.scalar if b == 0 else nc.sync
                eng.dma_start(ax[b, :, T // 2 :], out_b[:, T // 2 :])

    nc.compile()
    return nc


def kernel(x: np.ndarray, blocks: np.ndarray):
    """Full inputs in, full outputs out. Shards batch across 8 cores."""
    global LAST_RESULTS, _COMPILED
    if _COMPILED is None:
        _COMPILED = _build_module()
    nc = _COMPILED

    x = np.asarray(x, dtype=np.float32)
    blocks_np = np.ascontiguousarray(np.asarray(blocks, dtype=np.float32))
    if x.ndim == 4:
        x = x[:, 0]
    in_maps = [
        {
            "x": np.ascontiguousarray(x[k * B_SH : (k + 1) * B_SH]),
            "blocks": blocks_np,
        }
        for k in range(N_CORES)
    ]
    res = run_bass_kernel_spmd(nc, in_maps, core_ids=list(range(N_CORES)))
    LAST_RESULTS = res
    ax = np.concatenate([res.results[k]["ax"] for k in range(N_CORES)], axis=0)
    a_full = res.results[0]["afull"]
    return ax, a_full
